# revision 1
# baseline (speedup 1.0000x reference)
"""Trainium2 Bass kernel for nn_DeepVCP (retrieval_knn).

The reference computes a 5-layer 1x1-conv saliency MLP (6->32->64->16->8->1)
over batch 0 only, takes the top-64 point indices of the (softplus) saliency,
and gathers those columns from src_pts for ALL batches:
    out[b, k, c] = src_pts[b, c, idx_k],  idx = top_k(w[0,0], 64).
(The FPS/ball-query results in the reference are computed then discarded; the
final softplus + bias of the last conv are strictly monotone so the top-k of
the pre-activation logits is identical.)

Sharding: data-parallel over batch B across the 8 cores.  Keypoint selection
is replicated: every core computes the batch-0 saliency MLP + device-side
top-64 locally (no collectives), then gathers its own batch's points with an
indirect DMA.  Host only stacks the 8 per-core [64, 8] outputs.

MLP layout: the 65536 points are split into 4 chunks of 16384 laid across
partitions with block-diagonal weights, so each layer is a single (or two)
128-contraction matmul per 512-point stripe.

Top-64: 8 rounds of (per-partition max8 -> DMA flatten -> global max8 ->
PE-broadcast -> match_replace), then index recovery via max_index's
not-found sentinel + ones-matmul partition reduction.
"""

import numpy as np

import concourse.bass as bass
import concourse.tile as tile
from concourse import bacc, mybir
from concourse.bass_utils import run_bass_kernel_spmd

F32 = mybir.dt.float32
P = 128
N = 65536
NCHUNK = 4
M = N // NCHUNK          # 16384 points per chunk
FT = 512                 # stripe width (points per chunk per f-tile)
NFT = M // FT            # 32 f-tiles
K = 64
NEG = -1.0e30

_CACHE = {}


def _build_bass():
    nc = bacc.Bacc("TRN2", target_bir_lowering=False, debug=False, num_devices=8)

    # ---- DRAM I/O ----
    d_x0 = nc.dram_tensor("x0blk", [24, M], F32, kind="ExternalInput").ap()
    d_xg = nc.dram_tensor("xgT", [N, 8], F32, kind="ExternalInput").ap()
    d_s1 = nc.dram_tensor("S1", [24, 128], F32, kind="ExternalInput").ap()
    d_s2 = nc.dram_tensor("S2d", [128, 128], F32, kind="ExternalInput").ap()
    d_s3 = nc.dram_tensor("S3", [128, 32], F32, kind="ExternalInput").ap()
    d_s4 = nc.dram_tensor("S4", [64, 32], F32, kind="ExternalInput").ap()
    d_s5 = nc.dram_tensor("S5", [32, 4], F32, kind="ExternalInput").ap()
    d_b1 = nc.dram_tensor("b1v", [128, 1], F32, kind="ExternalInput").ap()
    d_b2 = nc.dram_tensor("b2v", [128, 1], F32, kind="ExternalInput").ap()
    d_b3 = nc.dram_tensor("b3v", [64, 1], F32, kind="ExternalInput").ap()
    d_b4 = nc.dram_tensor("b4v", [32, 1], F32, kind="ExternalInput").ap()
    d_ones1 = nc.dram_tensor("ones1", [1, 128], F32, kind="ExternalInput").ap()
    d_ones128 = nc.dram_tensor("ones128", [128, 1], F32, kind="ExternalInput").ap()
    d_pbase = nc.dram_tensor("pbase", [128, 1], F32, kind="ExternalInput").ap()
    d_out = nc.dram_tensor("out", [K, 8], F32, kind="ExternalOutput").ap()
    d_dbg = nc.dram_tensor("dbg", [2, K], F32, kind="ExternalOutput").ap()

    RELU = mybir.ActivationFunctionType.Relu

    with tile.TileContext(nc) as tc:
        with tc.tile_pool(name="cst", bufs=1) as cst, \
             tc.tile_pool(name="sb", bufs=1) as sb, \
             tc.tile_pool(name="ps", bufs=1, space="PSUM") as ps:

            def load_const(d_ap, shape, tag):
                t = cst.tile(shape, F32, tag=tag)
                nc.sync.dma_start(t[:], d_ap[:])
                return t

            x0 = load_const(d_x0, [24, M], "x0")
            s1 = load_const(d_s1, [24, 128], "s1")
            s2 = load_const(d_s2, [128, 128], "s2")
            s3 = load_const(d_s3, [128, 32], "s3")
            s4 = load_const(d_s4, [64, 32], "s4")
            s5 = load_const(d_s5, [32, 4], "s5")
            b1 = load_const(d_b1, [128, 1], "b1")
            b2 = load_const(d_b2, [128, 1], "b2")
            b3 = load_const(d_b3, [64, 1], "b3")
            b4 = load_const(d_b4, [32, 1], "b4")
            ones1 = load_const(d_ones1, [1, 128], "ones1")
            ones128 = load_const(d_ones128, [128, 1], "ones128")
            pbase = load_const(d_pbase, [128, 1], "pbase")

            zbig = cst.tile([NCHUNK, M], F32, tag="zbig")

            # ================= saliency MLP =================
            for f in range(NFT):
                fs = slice(f * FT, (f + 1) * FT)
                # L1: 6->32, 4 chunk-blocks
                p1 = ps.tile([128, FT], F32, tag="pbig")
                nc.tensor.matmul(p1[:], s1[:], x0[:, fs], start=True, stop=True)
                x2 = sb.tile([128, FT], F32, tag="x2")
                nc.scalar.activation(x2[:], p1[:], RELU, bias=b1[:, :1])
                # L2: 32->64, two chunk-pairs
                pA = ps.tile([128, FT], F32, tag="pbig")
                nc.tensor.matmul(pA[:], s2[0:64, :], x2[0:64, :], start=True, stop=True)
                pB = ps.tile([128, FT], F32, tag="pbig")
                nc.tensor.matmul(pB[:], s2[64:128, :], x2[64:128, :], start=True, stop=True)
                x3a = sb.tile([128, FT], F32, tag="x3")
                nc.scalar.activation(x3a[:], pA[:], RELU, bias=b2[:, :1])
                x3b = sb.tile([128, FT], F32, tag="x3")
                if f % 2 == 0:
                    nc.vector.tensor_scalar(x3b[:], pB[:], b2[:, :1], 0.0,
                                            op0=mybir.AluOpType.add,
                                            op1=mybir.AluOpType.max)
                else:
                    nc.scalar.activation(x3b[:], pB[:], RELU, bias=b2[:, :1])
                # L3: 64->16, 2 blocks per pair; psum quadrant placement
                p3 = ps.tile([64, FT], F32, tag="psmall")
                nc.tensor.matmul(p3[0:32, :], s3[:], x3a[:], start=True, stop=True)
                nc.tensor.matmul(p3[32:64, :], s3[:], x3b[:], start=True, stop=True)
                x4 = sb.tile([64, FT], F32, tag="x4")
                nc.vector.tensor_scalar(x4[:], p3[:], b3[:, :1], 0.0,
                                        op0=mybir.AluOpType.add,
                                        op1=mybir.AluOpType.max)
                # L4: 16->8, 4 blocks
                p4 = ps.tile([32, FT], F32, tag="psmall")
                nc.tensor.matmul(p4[:], s4[:], x4[:], start=True, stop=True)
                x5 = sb.tile([32, FT], F32, tag="x5")
                if f % 2 == 0:
                    nc.scalar.activation(x5[:], p4[:], RELU, bias=b4[:, :1])
                else:
                    nc.vector.tensor_scalar(x5[:], p4[:], b4[:, :1], 0.0,
                                            op0=mybir.AluOpType.add,
                                            op1=mybir.AluOpType.max)
                # L5: 8->1, 4 blocks (no bias/softplus: monotone, top-k invariant)
                p5 = ps.tile([NCHUNK, FT], F32, tag="psmall")
                nc.tensor.matmul(p5[:], s5[:], x5[:], start=True, stop=True)
                if f % 2 == 0:
                    nc.vector.tensor_copy(zbig[:, fs], p5[:])
                else:
                    nc.scalar.activation(zbig[:, fs], p5[:],
                                         mybir.ActivationFunctionType.Copy)

            # reshape [4, 16384] -> [128, 512]; Z[p, j] is point pbase[p] + j
            z = cst.tile([P, FT], F32, tag="z")
            nc.sync.dma_start(z[:], zbig[:])

            # ================= global top-64 =================
            zs = cst.tile([P, FT], F32, tag="zs")
            G = cst.tile([1, K], F32, tag="G")
            cur = z
            for r in range(8):
                v8 = sb.tile([P, 8], F32, tag="v8")
                nc.vector.max(out=v8[:], in_=cur[:])
                vf = sb.tile([1, 1024], F32, tag="vf")
                nc.sync.dma_start(vf[:], v8[:])
                g8 = sb.tile([1, 8], F32, tag="g8")
                nc.vector.max(out=g8[:], in_=vf[:])
                pb8 = ps.tile([P, 8], F32, tag="ptk")
                nc.tensor.matmul(pb8[:], ones1[:], g8[:], start=True, stop=True)
                b8 = sb.tile([P, 8], F32, tag="b8")
                nc.vector.tensor_copy(b8[:], pb8[:])
                nc.vector.match_replace(out=zs[:], in_to_replace=b8[:],
                                        in_values=cur[:], imm_value=NEG)
                cur = zs
                nc.scalar.activation(G[:, 8 * r:8 * r + 8], g8[:],
                                     mybir.ActivationFunctionType.Copy)

            # ---- index recovery (max_index sentinel + partition-sum) ----
            pG = ps.tile([P, K], F32, tag="ptk")
            nc.tensor.matmul(pG[:], ones1[:], G[:], start=True, stop=True)
            Gb = sb.tile([P, K], F32, tag="Gb")
            nc.vector.tensor_copy(Gb[:], pG[:])
            I = sb.tile([P, K], mybir.dt.uint32, tag="I")
            for b in range(8):
                nc.vector.max_index(out=I[:, 8 * b:8 * b + 8],
                                    in_max=Gb[:, 8 * b:8 * b + 8], in_values=z[:])
            If = sb.tile([P, K], F32, tag="If")
            nc.vector.tensor_copy(If[:], I[:])
            found = sb.tile([P, K], F32, tag="found")
            nc.vector.tensor_scalar(found[:], If[:], 1.0e6, None,
                                    op0=mybir.AluOpType.is_lt)
            lin = sb.tile([P, K], F32, tag="lin")
            nc.vector.tensor_scalar(lin[:], If[:], pbase[:, :1], None,
                                    op0=mybir.AluOpType.add)
            nc.vector.tensor_tensor(out=lin[:], in0=lin[:], in1=found[:],
                                    op=mybir.AluOpType.mult)
            pI = ps.tile([1, K], F32, tag="ptk")
            nc.tensor.matmul(pI[:], ones128[:], lin[:], start=True, stop=True)
            idxf = sb.tile([1, K], F32, tag="idxf")
            nc.vector.tensor_copy(idxf[:], pI[:])

            # ---- transpose [1,64] -> [64,1], cast int32, gather own batch ----
            one1 = sb.tile([1, 1], F32, tag="one1")
            nc.vector.memset(one1[:], 1.0)
            pT = ps.tile([K, 1], F32, tag="ptk")
            nc.tensor.matmul(pT[:], idxf[:], one1[:], start=True, stop=True)
            idx32 = sb.tile([K, 1], mybir.dt.int32, tag="idx32")
            nc.vector.tensor_copy(idx32[:], pT[:])
            gat = sb.tile([K, 8], F32, tag="gat")
            nc.gpsimd.indirect_dma_start(
                out=gat[:], out_offset=None, in_=d_xg[:],
                in_offset=bass.IndirectOffsetOnAxis(ap=idx32[:, :1], axis=0))

            nc.sync.dma_start(d_out[:], gat[:])
            nc.sync.dma_start(d_dbg[0:1, :], G[:])
            nc.sync.dma_start(d_dbg[1:2, :], idxf[:])

    nc.compile()
    return nc


def _host_prep(src_pts, W1, b1, W2, b2, Wa, ba, Wb, bb, Wc, bc):
    src = np.ascontiguousarray(np.asarray(src_pts, dtype=np.float32))
    B = src.shape[0]
    x0 = src[0]                                        # [6, 65536]
    x0blk = np.ascontiguousarray(
        x0.reshape(6, NCHUNK, M).transpose(1, 0, 2).reshape(24, M))

    W1, W2, Wa, Wb, Wc = (np.asarray(w, np.float32) for w in (W1, W2, Wa, Wb, Wc))
    b1, b2, ba, bb = (np.asarray(v, np.float32) for v in (b1, b2, ba, bb))

    S1 = np.zeros((24, 128), np.float32)
    for c in range(4):
        S1[6 * c:6 * c + 6, 32 * c:32 * c + 32] = W1.T
    S2d = np.zeros((128, 128), np.float32)
    for h in range(2):
        for a in range(2):
            S2d[64 * h + 32 * a:64 * h + 32 * a + 32, 64 * a:64 * a + 64] = W2.T
    S3 = np.zeros((128, 32), np.float32)
    for a in range(2):
        S3[64 * a:64 * a + 64, 16 * a:16 * a + 16] = Wa.T
    S4 = np.zeros((64, 32), np.float32)
    for c in range(4):
        S4[16 * c:16 * c + 16, 8 * c:8 * c + 8] = Wb.T
    S5 = np.zeros((32, 4), np.float32)
    for c in range(4):
        S5[8 * c:8 * c + 8, c:c + 1] = Wc.T

    b1v = np.tile(b1, 4)[:, None].astype(np.float32)
    b2v = np.tile(b2, 2)[:, None].astype(np.float32)
    b3v = np.tile(ba, 4)[:, None].astype(np.float32)
    b4v = np.tile(bb, 4)[:, None].astype(np.float32)

    pb = np.arange(P)
    pbase = ((pb // 32) * M + (pb % 32) * FT).astype(np.float32)[:, None]

    common = {
        "x0blk": x0blk, "S1": S1, "S2d": S2d, "S3": S3, "S4": S4, "S5": S5,
        "b1v": b1v, "b2v": b2v, "b3v": b3v, "b4v": b4v,
        "ones1": np.ones((1, 128), np.float32),
        "ones128": np.ones((128, 1), np.float32),
        "pbase": pbase,
    }
    in_maps = []
    for c in range(8):
        xgT = np.zeros((N, 8), np.float32)
        xgT[:, :6] = src[c % B].T
        in_maps.append(dict(common, xgT=xgT))
    return in_maps


def kernel(**inputs):
    if "nc" not in _CACHE:
        _CACHE["nc"] = _build_bass()
    nc = _CACHE["nc"]
    in_maps = _host_prep(
        inputs["src_pts"], inputs["W1"], inputs["b1"], inputs["W2"], inputs["b2"],
        inputs["Wa"], inputs["ba"], inputs["Wb"], inputs["bb"],
        inputs["Wc"], inputs["bc"])
    res = run_bass_kernel_spmd(nc, in_maps, core_ids=list(range(8)),
                               **_CACHE.get("run_kwargs", {}))
    _CACHE["last_results"] = res
    out = np.stack([res.results[c]["out"][:, :6] for c in range(8)], axis=0)
    return out.astype(np.float32)


# revision 2
# speedup vs baseline: 4.4267x; 4.4267x over previous
"""Trainium2 Bass kernel for nn_DeepVCP (retrieval_knn).

The reference computes a 5-layer 1x1-conv saliency MLP (6->32->64->16->8->1)
over batch 0 only, takes the top-64 point indices of the (softplus) saliency,
and gathers those columns from src_pts for ALL batches:
    out[b, k, c] = src_pts[b, c, idx_k],  idx = top_k(w[0,0], 64).
(The FPS/ball-query results in the reference are computed then discarded; the
final softplus + bias of the last conv are strictly monotone so the top-k of
the pre-activation logits is identical.)

Two SPMD launches over the 8 cores:

Phase A - saliency MLP, sharded over the 65536 points (each core computes the
  f32 logits for its 8192-point slice of batch 0; fp32 PE matmuls keep the
  scores bit-comparable to the reference).  Host concatenates the 8 slices
  (pure resharding, no arithmetic).

Phase B - replicated device-side top-64 (8 rounds of per-partition max8 ->
  DMA flatten -> global max8 -> PE broadcast -> match_replace), index
  recovery via max_index's not-found sentinel + ones-matmul partition
  reduction, then each core indirect-DMA-gathers its own batch's points.
  Host only stacks the per-core [64, 8] outputs.
"""

import numpy as np

import concourse.bass as bass
import concourse.tile as tile
from concourse import bacc, mybir
from concourse.bass_utils import run_bass_kernel_spmd

F32 = mybir.dt.float32
P = 128
N = 65536
NCORE = 8
NA = N // NCORE          # 8192 points per core in phase A
NCHUNK = 4
MA = NA // NCHUNK        # 2048 points per chunk
FT = 512
NFT = MA // FT           # 4 f-tiles per core
K = 64
NEG = -1.0e30

_CACHE = {}


def _build_phase_a():
    """Saliency MLP over this core's 8192-point slice -> z [4, 2048] f32."""
    nc = bacc.Bacc("TRN2", target_bir_lowering=False, debug=False, num_devices=NCORE)

    d_x0 = nc.dram_tensor("x0blk", [24, MA], F32, kind="ExternalInput").ap()
    d_s1 = nc.dram_tensor("S1", [24, 128], F32, kind="ExternalInput").ap()
    d_s2 = nc.dram_tensor("S2d", [128, 128], F32, kind="ExternalInput").ap()
    d_s3 = nc.dram_tensor("S3", [128, 32], F32, kind="ExternalInput").ap()
    d_s4 = nc.dram_tensor("S4", [64, 32], F32, kind="ExternalInput").ap()
    d_s5 = nc.dram_tensor("S5", [32, 4], F32, kind="ExternalInput").ap()
    d_b1 = nc.dram_tensor("b1v", [128, 1], F32, kind="ExternalInput").ap()
    d_b2 = nc.dram_tensor("b2v", [128, 1], F32, kind="ExternalInput").ap()
    d_b3 = nc.dram_tensor("b3v", [64, 1], F32, kind="ExternalInput").ap()
    d_b4 = nc.dram_tensor("b4v", [32, 1], F32, kind="ExternalInput").ap()
    d_z = nc.dram_tensor("z", [NCHUNK, MA], F32, kind="ExternalOutput").ap()

    RELU = mybir.ActivationFunctionType.Relu

    with tile.TileContext(nc) as tc:
        with tc.tile_pool(name="cst", bufs=1) as cst, \
             tc.tile_pool(name="sb", bufs=2) as sb, \
             tc.tile_pool(name="ps", bufs=2, space="PSUM") as ps:

            def load_const(d_ap, shape, tag):
                t = cst.tile(shape, F32, tag=tag)
                nc.sync.dma_start(t[:], d_ap[:])
                return t

            x0 = load_const(d_x0, [24, MA], "x0")
            s1 = load_const(d_s1, [24, 128], "s1")
            s2 = load_const(d_s2, [128, 128], "s2")
            s3 = load_const(d_s3, [128, 32], "s3")
            s4 = load_const(d_s4, [64, 32], "s4")
            s5 = load_const(d_s5, [32, 4], "s5")
            b1 = load_const(d_b1, [128, 1], "b1")
            b2 = load_const(d_b2, [128, 1], "b2")
            b3 = load_const(d_b3, [64, 1], "b3")
            b4 = load_const(d_b4, [32, 1], "b4")

            zbig = cst.tile([NCHUNK, MA], F32, tag="zbig")

            for f in range(NFT):
                fs = slice(f * FT, (f + 1) * FT)
                # L1: 6->32, 4 chunk-blocks
                p1 = ps.tile([128, FT], F32, tag="pbig")
                nc.tensor.matmul(p1[:], s1[:], x0[:, fs], start=True, stop=True)
                x2 = sb.tile([128, FT], F32, tag="x2")
                nc.scalar.activation(x2[:], p1[:], RELU, bias=b1[:, :1])
                # L2: 32->64, two chunk-pairs
                pA = ps.tile([128, FT], F32, tag="pbig")
                nc.tensor.matmul(pA[:], s2[0:64, :], x2[0:64, :], start=True, stop=True)
                pB = ps.tile([128, FT], F32, tag="pbig")
                nc.tensor.matmul(pB[:], s2[64:128, :], x2[64:128, :], start=True, stop=True)
                x3a = sb.tile([128, FT], F32, tag="x3")
                nc.scalar.activation(x3a[:], pA[:], RELU, bias=b2[:, :1])
                x3b = sb.tile([128, FT], F32, tag="x3")
                if f % 2 == 0:
                    nc.vector.tensor_scalar(x3b[:], pB[:], b2[:, :1], 0.0,
                                            op0=mybir.AluOpType.add,
                                            op1=mybir.AluOpType.max)
                else:
                    nc.scalar.activation(x3b[:], pB[:], RELU, bias=b2[:, :1])
                # L3: 64->16, 2 blocks per pair; psum quadrant placement
                p3 = ps.tile([64, FT], F32, tag="psmall")
                nc.tensor.matmul(p3[0:32, :], s3[:], x3a[:], start=True, stop=True)
                nc.tensor.matmul(p3[32:64, :], s3[:], x3b[:], start=True, stop=True)
                x4 = sb.tile([64, FT], F32, tag="x4")
                nc.vector.tensor_scalar(x4[:], p3[:], b3[:, :1], 0.0,
                                        op0=mybir.AluOpType.add,
                                        op1=mybir.AluOpType.max)
                # L4: 16->8, 4 blocks
                p4 = ps.tile([32, FT], F32, tag="psmall")
                nc.tensor.matmul(p4[:], s4[:], x4[:], start=True, stop=True)
                x5 = sb.tile([32, FT], F32, tag="x5")
                if f % 2 == 0:
                    nc.scalar.activation(x5[:], p4[:], RELU, bias=b4[:, :1])
                else:
                    nc.vector.tensor_scalar(x5[:], p4[:], b4[:, :1], 0.0,
                                            op0=mybir.AluOpType.add,
                                            op1=mybir.AluOpType.max)
                # L5: 8->1, 4 blocks (no bias/softplus: monotone, top-k invariant)
                p5 = ps.tile([NCHUNK, FT], F32, tag="psmall")
                nc.tensor.matmul(p5[:], s5[:], x5[:], start=True, stop=True)
                if f % 2 == 0:
                    nc.vector.tensor_copy(zbig[:, fs], p5[:])
                else:
                    nc.scalar.activation(zbig[:, fs], p5[:],
                                         mybir.ActivationFunctionType.Copy)

            nc.sync.dma_start(d_z[:], zbig[:])

    nc.compile()
    return nc


def _build_phase_b():
    """Replicated top-64 of z + per-core batch gather."""
    nc = bacc.Bacc("TRN2", target_bir_lowering=False, debug=False, num_devices=NCORE)

    d_zin = nc.dram_tensor("zfull", [P, FT], F32, kind="ExternalInput").ap()
    d_xg = nc.dram_tensor("xgT", [N, 8], F32, kind="ExternalInput").ap()
    d_ones1 = nc.dram_tensor("ones1", [1, 128], F32, kind="ExternalInput").ap()
    d_ones128 = nc.dram_tensor("ones128", [128, 1], F32, kind="ExternalInput").ap()
    d_pbase = nc.dram_tensor("pbase", [128, 1], F32, kind="ExternalInput").ap()
    d_out = nc.dram_tensor("out", [K, 8], F32, kind="ExternalOutput").ap()
    d_dbg = nc.dram_tensor("dbg", [2, K], F32, kind="ExternalOutput").ap()

    with tile.TileContext(nc) as tc:
        with tc.tile_pool(name="cst", bufs=1) as cst, \
             tc.tile_pool(name="sb", bufs=2) as sb, \
             tc.tile_pool(name="ps", bufs=2, space="PSUM") as ps:

            z = cst.tile([P, FT], F32, tag="z")
            nc.sync.dma_start(z[:], d_zin[:])
            ones1 = cst.tile([1, 128], F32, tag="ones1")
            nc.sync.dma_start(ones1[:], d_ones1[:])
            ones128 = cst.tile([128, 1], F32, tag="ones128")
            nc.sync.dma_start(ones128[:], d_ones128[:])
            pbase = cst.tile([128, 1], F32, tag="pbase")
            nc.sync.dma_start(pbase[:], d_pbase[:])

            zs = cst.tile([P, FT], F32, tag="zs")
            G = cst.tile([1, K], F32, tag="G")
            cur = z
            for r in range(8):
                v8 = sb.tile([P, 8], F32, tag="v8")
                nc.vector.max(out=v8[:], in_=cur[:])
                vf = sb.tile([1, 1024], F32, tag="vf")
                nc.sync.dma_start(vf[:], v8[:])
                g8 = sb.tile([1, 8], F32, tag="g8")
                nc.vector.max(out=g8[:], in_=vf[:])
                pb8 = ps.tile([P, 8], F32, tag="ptk")
                nc.tensor.matmul(pb8[:], ones1[:], g8[:], start=True, stop=True)
                b8 = sb.tile([P, 8], F32, tag="b8")
                nc.vector.tensor_copy(b8[:], pb8[:])
                nc.vector.match_replace(out=zs[:], in_to_replace=b8[:],
                                        in_values=cur[:], imm_value=NEG)
                cur = zs
                nc.scalar.activation(G[:, 8 * r:8 * r + 8], g8[:],
                                     mybir.ActivationFunctionType.Copy)

            # ---- index recovery (max_index sentinel + partition-sum) ----
            pG = ps.tile([P, K], F32, tag="ptk")
            nc.tensor.matmul(pG[:], ones1[:], G[:], start=True, stop=True)
            Gb = sb.tile([P, K], F32, tag="Gb")
            nc.vector.tensor_copy(Gb[:], pG[:])
            I = sb.tile([P, K], mybir.dt.uint32, tag="I")
            for b in range(8):
                nc.vector.max_index(out=I[:, 8 * b:8 * b + 8],
                                    in_max=Gb[:, 8 * b:8 * b + 8], in_values=z[:])
            If = sb.tile([P, K], F32, tag="If")
            nc.vector.tensor_copy(If[:], I[:])
            found = sb.tile([P, K], F32, tag="found")
            nc.vector.tensor_scalar(found[:], If[:], 1.0e6, None,
                                    op0=mybir.AluOpType.is_lt)
            lin = sb.tile([P, K], F32, tag="lin")
            nc.vector.tensor_scalar(lin[:], If[:], pbase[:, :1], None,
                                    op0=mybir.AluOpType.add)
            nc.vector.tensor_tensor(out=lin[:], in0=lin[:], in1=found[:],
                                    op=mybir.AluOpType.mult)
            pI = ps.tile([1, K], F32, tag="ptk")
            nc.tensor.matmul(pI[:], ones128[:], lin[:], start=True, stop=True)
            idxf = sb.tile([1, K], F32, tag="idxf")
            nc.vector.tensor_copy(idxf[:], pI[:])

            # ---- transpose [1,64] -> [64,1], cast int32, gather own batch ----
            one1 = sb.tile([1, 1], F32, tag="one1")
            nc.vector.memset(one1[:], 1.0)
            pT = ps.tile([K, 1], F32, tag="ptk")
            nc.tensor.matmul(pT[:], idxf[:], one1[:], start=True, stop=True)
            idx32 = sb.tile([K, 1], mybir.dt.int32, tag="idx32")
            nc.vector.tensor_copy(idx32[:], pT[:])
            gat = sb.tile([K, 8], F32, tag="gat")
            nc.gpsimd.indirect_dma_start(
                out=gat[:], out_offset=None, in_=d_xg[:],
                in_offset=bass.IndirectOffsetOnAxis(ap=idx32[:, :1], axis=0))

            nc.sync.dma_start(d_out[:], gat[:])
            nc.sync.dma_start(d_dbg[0:1, :], G[:])
            nc.sync.dma_start(d_dbg[1:2, :], idxf[:])

    nc.compile()
    return nc


def _host_prep_a(src_pts, W1, b1, W2, b2, Wa, ba, Wb, bb, Wc, bc):
    src = np.ascontiguousarray(np.asarray(src_pts, dtype=np.float32))
    x0 = src[0]                                        # [6, 65536]

    W1, W2, Wa, Wb, Wc = (np.asarray(w, np.float32) for w in (W1, W2, Wa, Wb, Wc))
    b1, b2, ba, bb = (np.asarray(v, np.float32) for v in (b1, b2, ba, bb))

    S1 = np.zeros((24, 128), np.float32)
    for c in range(4):
        S1[6 * c:6 * c + 6, 32 * c:32 * c + 32] = W1.T
    S2d = np.zeros((128, 128), np.float32)
    for h in range(2):
        for a in range(2):
            S2d[64 * h + 32 * a:64 * h + 32 * a + 32, 64 * a:64 * a + 64] = W2.T
    S3 = np.zeros((128, 32), np.float32)
    for a in range(2):
        S3[64 * a:64 * a + 64, 16 * a:16 * a + 16] = Wa.T
    S4 = np.zeros((64, 32), np.float32)
    for c in range(4):
        S4[16 * c:16 * c + 16, 8 * c:8 * c + 8] = Wb.T
    S5 = np.zeros((32, 4), np.float32)
    for c in range(4):
        S5[8 * c:8 * c + 8, c:c + 1] = Wc.T

    common = {
        "S1": S1, "S2d": S2d, "S3": S3, "S4": S4, "S5": S5,
        "b1v": np.tile(b1, 4)[:, None].astype(np.float32),
        "b2v": np.tile(b2, 2)[:, None].astype(np.float32),
        "b3v": np.tile(ba, 4)[:, None].astype(np.float32),
        "b4v": np.tile(bb, 4)[:, None].astype(np.float32),
    }
    in_maps = []
    for c in range(NCORE):
        sl = x0[:, c * NA:(c + 1) * NA]                # [6, 8192]
        x0blk = np.ascontiguousarray(
            sl.reshape(6, NCHUNK, MA).transpose(1, 0, 2).reshape(24, MA))
        in_maps.append(dict(common, x0blk=x0blk))
    return in_maps


def _host_prep_b(src_pts, z_full):
    src = np.ascontiguousarray(np.asarray(src_pts, dtype=np.float32))
    B = src.shape[0]
    pbase = (np.arange(P) * FT).astype(np.float32)[:, None]
    common = {
        "zfull": z_full.reshape(P, FT),
        "ones1": np.ones((1, 128), np.float32),
        "ones128": np.ones((128, 1), np.float32),
        "pbase": pbase,
    }
    in_maps = []
    for c in range(NCORE):
        xgT = np.zeros((N, 8), np.float32)
        xgT[:, :6] = src[c % B].T
        in_maps.append(dict(common, xgT=xgT))
    return in_maps


def kernel(**inputs):
    if "nc_a" not in _CACHE:
        _CACHE["nc_a"] = _build_phase_a()
    if "nc_b" not in _CACHE:
        _CACHE["nc_b"] = _build_phase_b()

    run_kwargs = _CACHE.get("run_kwargs", {})
    wargs = (inputs["W1"], inputs["b1"], inputs["W2"], inputs["b2"],
             inputs["Wa"], inputs["ba"], inputs["Wb"], inputs["bb"],
             inputs["Wc"], inputs["bc"])

    in_maps_a = _host_prep_a(inputs["src_pts"], *wargs)
    res_a = run_bass_kernel_spmd(_CACHE["nc_a"], in_maps_a,
                                 core_ids=list(range(NCORE)), **run_kwargs)
    _CACHE["res_a"] = res_a
    # resharding only: z_full[8192*c + 2048*b + t] = core c's z[b, t]
    z_full = np.concatenate(
        [np.asarray(res_a.results[c]["z"]).reshape(-1) for c in range(NCORE)])

    in_maps_b = _host_prep_b(inputs["src_pts"], z_full)
    res_b = run_bass_kernel_spmd(_CACHE["nc_b"], in_maps_b,
                                 core_ids=list(range(NCORE)), **run_kwargs)
    _CACHE["res_b"] = res_b
    _CACHE["last_results"] = res_b

    out = np.stack([res_b.results[c]["out"][:, :6] for c in range(NCORE)], axis=0)
    return out.astype(np.float32)


# revision 8
# speedup vs baseline: 5.3058x; 1.1986x over previous
"""Trainium2 Bass kernel for nn_DeepVCP (retrieval_knn).

The reference computes a 5-layer 1x1-conv saliency MLP (6->32->64->16->8->1)
over batch 0 only, takes the top-64 point indices of the (softplus) saliency,
and gathers those columns from src_pts for ALL batches:
    out[b, k, c] = src_pts[b, c, idx_k],  idx = top_k(w[0,0], 64).
(The FPS/ball-query results in the reference are computed then discarded; the
final softplus + bias of the last conv are strictly monotone so the top-k of
the pre-activation logits is identical.)

Two SPMD launches over the 8 cores:

Phase A - saliency MLP, sharded over the 65536 points (each core computes the
  f32 logits for its 8192-point slice of batch 0; fp32 PE matmuls keep the
  scores bit-comparable to the reference).  Host concatenates the 8 slices
  (pure resharding, no arithmetic).

Phase B - replicated device-side top-64 (8 rounds of per-partition max8 ->
  DMA flatten -> global max8 -> PE broadcast -> match_replace), index
  recovery via max_index's not-found sentinel + ones-matmul partition
  reduction, then each core indirect-DMA-gathers its own batch's points.
  Host only stacks the per-core [64, 8] outputs.
"""

import numpy as np

import concourse.bass as bass
import concourse.tile as tile
from concourse import bacc, mybir
from concourse.bass_utils import run_bass_kernel_spmd

F32 = mybir.dt.float32
P = 128
N = 65536
NCORE = 8
NA = N // NCORE          # 8192 points per core in phase A
NCHUNK = 4
MA = NA // NCHUNK        # 2048 points per chunk
FT = 512
NFT = MA // FT           # 4 f-tiles per core
K = 64
NEG = -1.0e30

_CACHE = {}


def _build_phase_a():
    """Saliency MLP over this core's 8192-point slice -> z [4, 2048] f32."""
    nc = bacc.Bacc("TRN2", target_bir_lowering=False, debug=False, num_devices=NCORE)

    d_x0 = nc.dram_tensor("x0blk", [24, MA], F32, kind="ExternalInput").ap()
    d_s1 = nc.dram_tensor("S1", [24, 128], F32, kind="ExternalInput").ap()
    d_s2 = nc.dram_tensor("S2d", [128, 128], F32, kind="ExternalInput").ap()
    d_s3 = nc.dram_tensor("S3", [128, 32], F32, kind="ExternalInput").ap()
    d_s4 = nc.dram_tensor("S4", [64, 32], F32, kind="ExternalInput").ap()
    d_s5 = nc.dram_tensor("S5", [32, 4], F32, kind="ExternalInput").ap()
    d_b1 = nc.dram_tensor("b1v", [128, 1], F32, kind="ExternalInput").ap()
    d_b2 = nc.dram_tensor("b2v", [128, 1], F32, kind="ExternalInput").ap()
    d_b3 = nc.dram_tensor("b3v", [64, 1], F32, kind="ExternalInput").ap()
    d_b4 = nc.dram_tensor("b4v", [32, 1], F32, kind="ExternalInput").ap()
    d_z = nc.dram_tensor("z", [NCHUNK, MA], F32, kind="ExternalOutput").ap()

    RELU = mybir.ActivationFunctionType.Relu

    with tile.TileContext(nc) as tc:
        with tc.tile_pool(name="cst", bufs=1) as cst, \
             tc.tile_pool(name="sb", bufs=3) as sb, \
             tc.tile_pool(name="ps", bufs=3, space="PSUM") as ps:

            def load_const(d_ap, shape, tag):
                t = cst.tile(shape, F32, tag=tag)
                nc.sync.dma_start(t[:], d_ap[:])
                return t

            x0 = load_const(d_x0, [24, MA], "x0")
            s1 = load_const(d_s1, [24, 128], "s1")
            s2 = load_const(d_s2, [128, 128], "s2")
            s3 = load_const(d_s3, [128, 32], "s3")
            s4 = load_const(d_s4, [64, 32], "s4")
            s5 = load_const(d_s5, [32, 4], "s5")
            b1 = load_const(d_b1, [128, 1], "b1")
            b2 = load_const(d_b2, [128, 1], "b2")
            b3 = load_const(d_b3, [64, 1], "b3")
            b4 = load_const(d_b4, [32, 1], "b4")

            zbig = cst.tile([NCHUNK, MA], F32, tag="zbig")

            for f in range(NFT):
                fs = slice(f * FT, (f + 1) * FT)
                # L1: 6->32, 4 chunk-blocks
                p1 = ps.tile([128, FT], F32, tag="pbig")
                nc.tensor.matmul(p1[:], s1[:], x0[:, fs], start=True, stop=True)
                x2 = sb.tile([128, FT], F32, tag="x2")
                nc.scalar.activation(x2[:], p1[:], RELU, bias=b1[:, :1])
                # L2: 32->64, two chunk-pairs
                pA = ps.tile([128, FT], F32, tag="pbig")
                nc.tensor.matmul(pA[:], s2[0:64, :], x2[0:64, :], start=True, stop=True)
                pB = ps.tile([128, FT], F32, tag="pbig")
                nc.tensor.matmul(pB[:], s2[64:128, :], x2[64:128, :], start=True, stop=True)
                x3a = sb.tile([128, FT], F32, tag="x3")
                nc.scalar.activation(x3a[:], pA[:], RELU, bias=b2[:, :1])
                x3b = sb.tile([128, FT], F32, tag="x3")
                if f % 2 == 0:
                    nc.vector.tensor_scalar(x3b[:], pB[:], b2[:, :1], 0.0,
                                            op0=mybir.AluOpType.add,
                                            op1=mybir.AluOpType.max)
                else:
                    nc.scalar.activation(x3b[:], pB[:], RELU, bias=b2[:, :1])
                # L3: 64->16, 2 blocks per pair; psum quadrant placement
                p3 = ps.tile([64, FT], F32, tag="psmall")
                nc.tensor.matmul(p3[0:32, :], s3[:], x3a[:], start=True, stop=True)
                nc.tensor.matmul(p3[32:64, :], s3[:], x3b[:], start=True, stop=True)
                x4 = sb.tile([64, FT], F32, tag="x4")
                nc.vector.tensor_scalar(x4[:], p3[:], b3[:, :1], 0.0,
                                        op0=mybir.AluOpType.add,
                                        op1=mybir.AluOpType.max)
                # L4: 16->8, 4 blocks
                p4 = ps.tile([32, FT], F32, tag="psmall")
                nc.tensor.matmul(p4[:], s4[:], x4[:], start=True, stop=True)
                x5 = sb.tile([32, FT], F32, tag="x5")
                if f % 2 == 0:
                    nc.scalar.activation(x5[:], p4[:], RELU, bias=b4[:, :1])
                else:
                    nc.vector.tensor_scalar(x5[:], p4[:], b4[:, :1], 0.0,
                                            op0=mybir.AluOpType.add,
                                            op1=mybir.AluOpType.max)
                # L5: 8->1, 4 blocks (no bias/softplus: monotone, top-k invariant)
                p5 = ps.tile([NCHUNK, FT], F32, tag="psmall")
                nc.tensor.matmul(p5[:], s5[:], x5[:], start=True, stop=True)
                if f % 2 == 0:
                    nc.vector.tensor_copy(zbig[:, fs], p5[:])
                else:
                    nc.scalar.activation(zbig[:, fs], p5[:],
                                         mybir.ActivationFunctionType.Copy)

            nc.sync.dma_start(d_z[:], zbig[:])

    nc.compile()
    return nc


def _build_phase_b_fast():
    """Replicated top-64 of z + per-core batch gather (fast path).

    Exact when no 512-point row of z holds >= 9 of the global top-64 (checked
    on device: cnt output must equal 64; host falls back to the slow path
    otherwise).  Candidates = per-partition top-8 (1024 values), merged with
    8 rounds of max8+match_replace on a single flattened [1, 1024] row.
    """
    nc = bacc.Bacc("TRN2", target_bir_lowering=False, debug=False, num_devices=NCORE)

    d_zin = nc.dram_tensor("zfull", [P, FT], F32, kind="ExternalInput").ap()
    d_xg = nc.dram_tensor("xgT", [N, 8], F32, kind="ExternalInput").ap()
    d_cst = nc.dram_tensor("cst", [P, 130], F32, kind="ExternalInput").ap()
    d_out = nc.dram_tensor("out", [K, 8], F32, kind="ExternalOutput").ap()
    d_dbg = nc.dram_tensor("dbg", [3, K], F32, kind="ExternalOutput").ap()

    with tile.TileContext(nc) as tc:
        with tc.tile_pool(name="cst", bufs=1) as cst, \
             tc.tile_pool(name="sb", bufs=2) as sb, \
             tc.tile_pool(name="ps", bufs=2, space="PSUM") as ps:

            z = cst.tile([P, FT], F32, tag="z")
            nc.sync.dma_start(z[:], d_zin[:])
            cpack = cst.tile([P, 130], F32, tag="cpack")
            nc.sync.dma_start(cpack[:], d_cst[:])
            ones128 = cpack[:, 0:1]
            pbase = cpack[:, 1:2]
            ones1 = cpack[0:1, 2:130]

            # per-partition top-8 -> flatten to one row of 1024 candidates
            v8 = cst.tile([P, 8], F32, tag="v8")
            nc.vector.max(out=v8[:], in_=z[:])
            vf = cst.tile([1, 1024], F32, tag="vf")
            nc.sync.dma_start(vf[:], v8[:])

            vw = cst.tile([1, 1024], F32, tag="vw")
            G = cst.tile([1, K], F32, tag="G")
            cur = vf
            for r in range(8):
                g8 = sb.tile([1, 8], F32, tag="g8")
                nc.vector.max(out=g8[:], in_=cur[:])
                nc.vector.match_replace(out=vw[:], in_to_replace=g8[:],
                                        in_values=cur[:], imm_value=NEG)
                cur = vw
                nc.scalar.activation(G[:, 8 * r:8 * r + 8], g8[:],
                                     mybir.ActivationFunctionType.Copy)

            # ---- index recovery (max_index sentinel + partition-sum) ----
            pG = ps.tile([P, K], F32, tag="ptk")
            nc.tensor.matmul(pG[:], ones1, G[:], start=True, stop=True)
            Gb = sb.tile([P, K], F32, tag="Gb")
            nc.vector.tensor_copy(Gb[:], pG[:])
            I = sb.tile([P, K], mybir.dt.uint32, tag="I")
            for b in range(8):
                nc.vector.max_index(out=I[:, 8 * b:8 * b + 8],
                                    in_max=Gb[:, 8 * b:8 * b + 8], in_values=z[:])
            If = sb.tile([P, K], F32, tag="If")
            nc.vector.tensor_copy(If[:], I[:])
            found = sb.tile([P, K], F32, tag="found")
            nc.vector.tensor_scalar(found[:], If[:], 1.0e6, None,
                                    op0=mybir.AluOpType.is_lt)
            lin = sb.tile([P, K], F32, tag="lin")
            nc.vector.tensor_scalar(lin[:], If[:], pbase, None,
                                    op0=mybir.AluOpType.add)
            nc.vector.tensor_tensor(out=lin[:], in0=lin[:], in1=found[:],
                                    op=mybir.AluOpType.mult)
            pI = ps.tile([1, K], F32, tag="ptk")
            nc.tensor.matmul(pI[:], ones128, lin[:], start=True, stop=True)
            idxf = sb.tile([1, K], F32, tag="idxf")
            nc.vector.tensor_copy(idxf[:], pI[:])

            # ---- exactness check: cnt = #{z >= G[63]} (must be 64) ----
            cmp = sb.tile([P, FT], F32, tag="cmp")
            nc.scalar.activation(cmp[:], z[:], mybir.ActivationFunctionType.Sign,
                                 bias=Gb[:, 63:64], scale=-1.0)
            # sign(-z + G63): -1 where z > G63, 0 where ==, +1 where z < G63
            cntp = sb.tile([P, 1], F32, tag="cntp")
            nc.vector.tensor_reduce(cntp[:], cmp[:], mybir.AxisListType.X,
                                    mybir.AluOpType.add)
            pC = ps.tile([1, 1], F32, tag="ptk")
            nc.tensor.matmul(pC[:], ones128, cntp[:], start=True, stop=True)
            cntf = sb.tile([1, 1], F32, tag="cntf")
            nc.vector.tensor_copy(cntf[:], pC[:])

            # ---- transpose [1,64] -> [64,1], cast int32, gather own batch ----
            one1 = sb.tile([1, 1], F32, tag="one1")
            nc.vector.memset(one1[:], 1.0)
            pT = ps.tile([K, 1], F32, tag="ptk")
            nc.tensor.matmul(pT[:], idxf[:], one1[:], start=True, stop=True)
            idx32 = sb.tile([K, 1], mybir.dt.int32, tag="idx32")
            nc.vector.tensor_copy(idx32[:], pT[:])
            gat = sb.tile([K, 8], F32, tag="gat")
            nc.gpsimd.indirect_dma_start(
                out=gat[:], out_offset=None, in_=d_xg[:],
                in_offset=bass.IndirectOffsetOnAxis(ap=idx32[:, :1], axis=0))

            nc.sync.dma_start(d_out[:], gat[:])
            nc.sync.dma_start(d_dbg[0:1, :], G[:])
            nc.sync.dma_start(d_dbg[1:2, :], idxf[:])
            nc.sync.dma_start(d_dbg[2:3, 0:1], cntf[:])

    nc.compile()
    return nc


def _build_phase_b():
    """Replicated top-64 of z + per-core batch gather."""
    nc = bacc.Bacc("TRN2", target_bir_lowering=False, debug=False, num_devices=NCORE)

    d_zin = nc.dram_tensor("zfull", [P, FT], F32, kind="ExternalInput").ap()
    d_xg = nc.dram_tensor("xgT", [N, 8], F32, kind="ExternalInput").ap()
    d_ones1 = nc.dram_tensor("ones1", [1, 128], F32, kind="ExternalInput").ap()
    d_ones128 = nc.dram_tensor("ones128", [128, 1], F32, kind="ExternalInput").ap()
    d_pbase = nc.dram_tensor("pbase", [128, 1], F32, kind="ExternalInput").ap()
    d_out = nc.dram_tensor("out", [K, 8], F32, kind="ExternalOutput").ap()
    d_dbg = nc.dram_tensor("dbg", [2, K], F32, kind="ExternalOutput").ap()

    with tile.TileContext(nc) as tc:
        with tc.tile_pool(name="cst", bufs=1) as cst, \
             tc.tile_pool(name="sb", bufs=2) as sb, \
             tc.tile_pool(name="ps", bufs=2, space="PSUM") as ps:

            z = cst.tile([P, FT], F32, tag="z")
            nc.sync.dma_start(z[:], d_zin[:])
            ones1 = cst.tile([1, 128], F32, tag="ones1")
            nc.sync.dma_start(ones1[:], d_ones1[:])
            ones128 = cst.tile([128, 1], F32, tag="ones128")
            nc.sync.dma_start(ones128[:], d_ones128[:])
            pbase = cst.tile([128, 1], F32, tag="pbase")
            nc.sync.dma_start(pbase[:], d_pbase[:])

            zs = cst.tile([P, FT], F32, tag="zs")
            G = cst.tile([1, K], F32, tag="G")
            cur = z
            for r in range(8):
                v8 = sb.tile([P, 8], F32, tag="v8")
                nc.vector.max(out=v8[:], in_=cur[:])
                vf = sb.tile([1, 1024], F32, tag="vf")
                nc.sync.dma_start(vf[:], v8[:])
                g8 = sb.tile([1, 8], F32, tag="g8")
                nc.vector.max(out=g8[:], in_=vf[:])
                pb8 = ps.tile([P, 8], F32, tag="ptk")
                nc.tensor.matmul(pb8[:], ones1[:], g8[:], start=True, stop=True)
                b8 = sb.tile([P, 8], F32, tag="b8")
                nc.vector.tensor_copy(b8[:], pb8[:])
                nc.vector.match_replace(out=zs[:], in_to_replace=b8[:],
                                        in_values=cur[:], imm_value=NEG)
                cur = zs
                nc.scalar.activation(G[:, 8 * r:8 * r + 8], g8[:],
                                     mybir.ActivationFunctionType.Copy)

            # ---- index recovery (max_index sentinel + partition-sum) ----
            pG = ps.tile([P, K], F32, tag="ptk")
            nc.tensor.matmul(pG[:], ones1[:], G[:], start=True, stop=True)
            Gb = sb.tile([P, K], F32, tag="Gb")
            nc.vector.tensor_copy(Gb[:], pG[:])
            I = sb.tile([P, K], mybir.dt.uint32, tag="I")
            for b in range(8):
                nc.vector.max_index(out=I[:, 8 * b:8 * b + 8],
                                    in_max=Gb[:, 8 * b:8 * b + 8], in_values=z[:])
            If = sb.tile([P, K], F32, tag="If")
            nc.vector.tensor_copy(If[:], I[:])
            found = sb.tile([P, K], F32, tag="found")
            nc.vector.tensor_scalar(found[:], If[:], 1.0e6, None,
                                    op0=mybir.AluOpType.is_lt)
            lin = sb.tile([P, K], F32, tag="lin")
            nc.vector.tensor_scalar(lin[:], If[:], pbase[:, :1], None,
                                    op0=mybir.AluOpType.add)
            nc.vector.tensor_tensor(out=lin[:], in0=lin[:], in1=found[:],
                                    op=mybir.AluOpType.mult)
            pI = ps.tile([1, K], F32, tag="ptk")
            nc.tensor.matmul(pI[:], ones128[:], lin[:], start=True, stop=True)
            idxf = sb.tile([1, K], F32, tag="idxf")
            nc.vector.tensor_copy(idxf[:], pI[:])

            # ---- transpose [1,64] -> [64,1], cast int32, gather own batch ----
            one1 = sb.tile([1, 1], F32, tag="one1")
            nc.vector.memset(one1[:], 1.0)
            pT = ps.tile([K, 1], F32, tag="ptk")
            nc.tensor.matmul(pT[:], idxf[:], one1[:], start=True, stop=True)
            idx32 = sb.tile([K, 1], mybir.dt.int32, tag="idx32")
            nc.vector.tensor_copy(idx32[:], pT[:])
            gat = sb.tile([K, 8], F32, tag="gat")
            nc.gpsimd.indirect_dma_start(
                out=gat[:], out_offset=None, in_=d_xg[:],
                in_offset=bass.IndirectOffsetOnAxis(ap=idx32[:, :1], axis=0))

            nc.sync.dma_start(d_out[:], gat[:])
            nc.sync.dma_start(d_dbg[0:1, :], G[:])
            nc.sync.dma_start(d_dbg[1:2, :], idxf[:])

    nc.compile()
    return nc


def _host_prep_a(src_pts, W1, b1, W2, b2, Wa, ba, Wb, bb, Wc, bc):
    src = np.ascontiguousarray(np.asarray(src_pts, dtype=np.float32))
    x0 = src[0]                                        # [6, 65536]

    W1, W2, Wa, Wb, Wc = (np.asarray(w, np.float32) for w in (W1, W2, Wa, Wb, Wc))
    b1, b2, ba, bb = (np.asarray(v, np.float32) for v in (b1, b2, ba, bb))

    S1 = np.zeros((24, 128), np.float32)
    for c in range(4):
        S1[6 * c:6 * c + 6, 32 * c:32 * c + 32] = W1.T
    S2d = np.zeros((128, 128), np.float32)
    for h in range(2):
        for a in range(2):
            S2d[64 * h + 32 * a:64 * h + 32 * a + 32, 64 * a:64 * a + 64] = W2.T
    S3 = np.zeros((128, 32), np.float32)
    for a in range(2):
        S3[64 * a:64 * a + 64, 16 * a:16 * a + 16] = Wa.T
    S4 = np.zeros((64, 32), np.float32)
    for c in range(4):
        S4[16 * c:16 * c + 16, 8 * c:8 * c + 8] = Wb.T
    S5 = np.zeros((32, 4), np.float32)
    for c in range(4):
        S5[8 * c:8 * c + 8, c:c + 1] = Wc.T

    common = {
        "S1": S1, "S2d": S2d, "S3": S3, "S4": S4, "S5": S5,
        "b1v": np.tile(b1, 4)[:, None].astype(np.float32),
        "b2v": np.tile(b2, 2)[:, None].astype(np.float32),
        "b3v": np.tile(ba, 4)[:, None].astype(np.float32),
        "b4v": np.tile(bb, 4)[:, None].astype(np.float32),
    }
    in_maps = []
    for c in range(NCORE):
        sl = x0[:, c * NA:(c + 1) * NA]                # [6, 8192]
        x0blk = np.ascontiguousarray(
            sl.reshape(6, NCHUNK, MA).transpose(1, 0, 2).reshape(24, MA))
        in_maps.append(dict(common, x0blk=x0blk))
    return in_maps


def _xgt_maps(src_pts, common):
    src = np.ascontiguousarray(np.asarray(src_pts, dtype=np.float32))
    B = src.shape[0]
    in_maps = []
    for c in range(NCORE):
        xgT = np.zeros((N, 8), np.float32)
        xgT[:, :6] = src[c % B].T
        in_maps.append(dict(common, xgT=xgT))
    return in_maps


def _host_prep_b_fast(src_pts, z_full):
    cpack = np.zeros((P, 130), np.float32)
    cpack[:, 0] = 1.0
    cpack[:, 1] = np.arange(P) * FT
    cpack[0, 2:130] = 1.0
    return _xgt_maps(src_pts, {"zfull": z_full.reshape(P, FT), "cst": cpack})


def _host_prep_b_slow(src_pts, z_full):
    common = {
        "zfull": z_full.reshape(P, FT),
        "ones1": np.ones((1, 128), np.float32),
        "ones128": np.ones((128, 1), np.float32),
        "pbase": (np.arange(P) * FT).astype(np.float32)[:, None],
    }
    return _xgt_maps(src_pts, common)


# sum over z of sign(G63 - z) when the fast path's 64th value is exact:
# (N - 64) elements below + 63 above + 1 equal
_CNT_EXPECT = float((N - 64) - 63)


def kernel(**inputs):
    if "nc_a" not in _CACHE:
        _CACHE["nc_a"] = _build_phase_a()
    if "nc_bf" not in _CACHE:
        _CACHE["nc_bf"] = _build_phase_b_fast()

    run_kwargs = _CACHE.get("run_kwargs", {})
    wargs = (inputs["W1"], inputs["b1"], inputs["W2"], inputs["b2"],
             inputs["Wa"], inputs["ba"], inputs["Wb"], inputs["bb"],
             inputs["Wc"], inputs["bc"])

    in_maps_a = _host_prep_a(inputs["src_pts"], *wargs)
    res_a = run_bass_kernel_spmd(_CACHE["nc_a"], in_maps_a,
                                 core_ids=list(range(NCORE)), **run_kwargs)
    _CACHE["res_a"] = res_a
    # resharding only: z_full[8192*c + 2048*b + t] = core c's z[b, t]
    z_full = np.concatenate(
        [np.asarray(res_a.results[c]["z"]).reshape(-1) for c in range(NCORE)])

    in_maps_b = _host_prep_b_fast(inputs["src_pts"], z_full)
    res_b = run_bass_kernel_spmd(_CACHE["nc_bf"], in_maps_b,
                                 core_ids=list(range(NCORE)), **run_kwargs)
    _CACHE["res_b"] = res_b
    _CACHE["last_results"] = res_b

    cnt = float(res_b.results[0]["dbg"][2, 0])
    if cnt != _CNT_EXPECT:
        # >= 9 of the top-64 landed in one 512-point row: candidates were
        # incomplete.  Exact slow path (never taken for generic inputs).
        if "nc_bs" not in _CACHE:
            _CACHE["nc_bs"] = _build_phase_b()
        res_b = run_bass_kernel_spmd(_CACHE["nc_bs"],
                                     _host_prep_b_slow(inputs["src_pts"], z_full),
                                     core_ids=list(range(NCORE)), **run_kwargs)
        _CACHE["res_b"] = res_b
        _CACHE["last_results"] = res_b

    out = np.stack([res_b.results[c]["out"][:, :6] for c in range(NCORE)], axis=0)
    return out.astype(np.float32)
